# revision 21
# baseline (speedup 1.0000x reference)
"""Multi-head attention (B=2, N=2048, C=1024, H=16, D=64) on 8 TRN2 cores.

Sharding: tensor-parallel over heads — 2 heads per core. Each core computes
Q/K/V projections for its 2 heads, attention, and a partial output
projection (its heads' slice of Wo). Host sums the 8 partial outputs + bo.

Per-core dataflow (all matmul inputs bf16, PSUM accumulation fp32):
  xT [1024, 4096] (x transposed on host, replicated to all cores)
  QT/KT = W.T @ x.T   -> [128 (2 heads x 64), 4096]  (lhsT=W chunk, rhs=xT)
  VT likewise, then PE-transposed into v_aug [keys, 65] per head
  (65th column = ones -> softmax denominator comes out of the ctx matmul)
  S^T = K @ Q.T  -> [keys, q] in PSUM; exp on ScalarE -> bf16 SBUF
  ctx^T_aug [65, q] = v_aug.T @ expS^T  (row 64 = denominator)
  normalize: recip(row 64), gpsimd partition_broadcast, DVE multiply
  out_partial [4096, 1024] = ctx^T.T @ Wo_slice  (fp32 out, summed on host)

The 1/sqrt(D) scale is folded into Wq/bq on the host (exact: 0.125).
"""

import numpy as np
import ml_dtypes

import concourse.bass as bass
from concourse import bacc
import concourse.tile as tile
from concourse import mybir, library_config
from concourse.bass_utils import run_bass_kernel_spmd

BF16 = mybir.dt.bfloat16
F32 = mybir.dt.float32

B, N, C = 2, 2048, 1024
H, D = 16, 64
T = B * N              # 4096 tokens
HPC = H // 8           # heads per core = 2
DPC = HPC * D          # head dims per core = 128


def build_core_program(nc):
    """Emit the per-core SPMD program. Same program on all 8 cores;
    per-core data differences come from the input maps."""
    xT = nc.dram_tensor("xT", [C, T], BF16, kind="ExternalInput").ap()
    wq = nc.dram_tensor("wq", [C, DPC], BF16, kind="ExternalInput").ap()
    wk = nc.dram_tensor("wk", [C, DPC], BF16, kind="ExternalInput").ap()
    wv = nc.dram_tensor("wv", [C, DPC], BF16, kind="ExternalInput").ap()
    wo = nc.dram_tensor("wo", [DPC, C], BF16, kind="ExternalInput").ap()
    bqkv = nc.dram_tensor("bqkv", [DPC, 3], F32, kind="ExternalInput").ap()
    iden = nc.dram_tensor("iden", [128, 128], BF16, kind="ExternalInput").ap()
    out = nc.dram_tensor("out", [T, C], F32, kind="ExternalOutput").ap()

    KCH = C // 128     # 8 contraction chunks for projections
    NCH = T // 512     # 8 token chunks of 512
    KT16 = N // 128    # 16 key tiles per batch

    with tile.TileContext(nc) as tc:
        with tc.tile_pool(name="singles", bufs=1) as singles:
            nc.gpsimd.load_library(library_config.proxy)

            id_sb = singles.tile([128, 128], BF16, tag="iden")
            nc.sync.dma_start(out=id_sb, in_=iden)

            w_sb = {}
            for nm, w in (("wq", wq), ("wk", wk), ("wv", wv)):
                t = singles.tile([128, KCH, DPC], BF16, tag=f"w{nm}",
                                 name=f"w{nm}")
                nc.sync.dma_start(
                    out=t, in_=w.rearrange("(k p) j -> p k j", p=128))
                w_sb[nm] = [t[:, k, :] for k in range(KCH)]
            wo_sb = singles.tile([DPC, C], BF16, tag="wo")
            nc.sync.dma_start(out=wo_sb, in_=wo)

            bqkv_sb = singles.tile([DPC, 3], F32, tag="bqkv")
            nc.sync.dma_start(out=bqkv_sb, in_=bqkv)
            b_sb = {"bq": bqkv_sb[:, 0:1], "bk": bqkv_sb[:, 1:2],
                    "bv": bqkv_sb[:, 2:3]}

            # xT resident in SBUF as 8x2 tiles [128, 2048]; chunk 0 for all
            # feature rows first so the first projection starts after ~4MB.
            xt = [[singles.tile([128, 2048], BF16, tag=f"xt{k}_{c}", name=f"xt{k}_{c}")
                   for c in range(2)] for k in range(KCH)]
            for c in range(2):
                for k in range(KCH):
                    nc.sync.dma_start(
                        out=xt[k][c],
                        in_=xT[k * 128:(k + 1) * 128, c * 2048:(c + 1) * 2048])

            QT = singles.tile([128, T], BF16, tag="QT")
            KTt = singles.tile([128, T], BF16, tag="KT")
            VT = singles.tile([128, T], BF16, tag="VT")
            ctxTn = singles.tile([128, T], BF16, tag="ctxTn")
            vaug = [[singles.tile([128, KT16, D + 1], BF16, tag=f"vaug{b}{h}", name=f"vaug{b}{h}")
                     for h in range(HPC)] for b in range(B)]
            for b in range(B):
                for h in range(HPC):
                    nc.vector.memset(vaug[b][h], 1.0)

            # One unified PSUM layout for the whole kernel so projections and
            # attention can overlap freely (8 banks: pj 2 + s 4 + ctx 2).
            # Emission order interleaves per-batch: V+transposes and K for a
            # batch, then per q-chunk the matching Q projection followed by
            # that chunk's attention — later projections fill PE bubbles
            # while ACT works through the exps.
            with tc.tile_pool(name="psP", bufs=1, space="PSUM") as psP, \
                    tc.tile_pool(name="psO", bufs=1, space="PSUM") as psO, \
                    tc.tile_pool(name="psS", bufs=2, space="PSUM") as psS, \
                    tc.tile_pool(name="psC", bufs=1, space="PSUM") as psC, \
                    tc.tile_pool(name="esb", bufs=3) as esb, \
                    tc.tile_pool(name="nrm", bufs=2) as nrm, \
                    tc.tile_pool(name="csb", bufs=2) as csb, \
                    tc.tile_pool(name="osb", bufs=3) as osb:

                # keep PE busy (HAM warm) while xT streams in
                for wu in range(16):
                    ptw = psS.tile([128, 128], BF16, tag="s", name="ptw")
                    nc.tensor.transpose(ptw, id_sb, id_sb)
                proj_ctr = [0]

                def emit_proj(nm, dstT, nch, act_bias=False):
                    # alternate between the two single-bank tags so one
                    # chain's bias drain overlaps the next chain's matmuls
                    pool, tg = ((psP, "pj") if proj_ctr[0] % 2 == 0
                                else (psO, "po"))
                    proj_ctr[0] += 1
                    ps = pool.tile([128, 512], F32, tag=tg, name="pj")
                    c, off = divmod(nch * 512, 2048)
                    for k in range(KCH):
                        nc.tensor.matmul(
                            out=ps, lhsT=w_sb[nm][k],
                            rhs=xt[k][c][:, off:off + 512],
                            start=(k == 0), stop=(k == KCH - 1))
                    dst = dstT[:, nch * 512:(nch + 1) * 512]
                    if act_bias:
                        # ScalarE is idle before attention starts; using it
                        # for the bias keeps the single pj bank turning over
                        # without waiting on the DVE queue.
                        nc.scalar.activation(
                            out=dst, in_=ps,
                            func=mybir.ActivationFunctionType.Identity,
                            bias=b_sb["b" + nm[1]], scale=1.0)
                    else:
                        nc.vector.tensor_scalar_add(
                            out=dst, in0=ps, scalar1=b_sb["b" + nm[1]])
                    if nm == "wv":
                        # transpose the 4 just-projected 128-token tiles of V
                        # into v_aug [keys, 65] per head
                        for t16 in range(nch * 4, nch * 4 + 4):
                            b, bt = divmod(t16, KT16)
                            pt = psO.tile([128, 128], BF16, tag="po",
                                          name="pt")
                            base = t16 * 128
                            nc.tensor.transpose(
                                pt, VT[:, base:base + 128], id_sb)
                            nc.vector.tensor_copy(
                                out=vaug[b][0][:, bt, 0:D], in_=pt[:, 0:D])
                            nc.vector.tensor_copy(
                                out=vaug[b][1][:, bt, 0:D], in_=pt[:, D:2 * D])

                def emit_attention(b, qch, last=False):
                    q0 = b * N + qch * 512
                    ctx = [psC.tile([D + 1, 512], F32, tag=f"ctx{h}",
                                    name=f"ctx{h}") for h in range(HPC)]
                    for kc in range(KT16):
                        k0 = b * N + kc * 128
                        pS = psS.tile([128, 1024], F32, tag="s", name="s")
                        for h in range(HPC):
                            nc.tensor.matmul(
                                out=pS[:, h * 512:(h + 1) * 512],
                                lhsT=KTt[h * D:(h + 1) * D, k0:k0 + 128],
                                rhs=QT[h * D:(h + 1) * D, q0:q0 + 512],
                                start=True, stop=True)
                        eS = esb.tile([128, 1024], BF16, tag="e", name="e")
                        nc.scalar.activation(
                            eS, pS, mybir.ActivationFunctionType.Exp)
                        for h in range(HPC):
                            nc.tensor.matmul(
                                out=ctx[h],
                                lhsT=vaug[b][h][:, kc, :],
                                rhs=eS[:, h * 512:(h + 1) * 512],
                                start=(kc == 0), stop=(kc == KT16 - 1))
                    # normalize: rows 0..63 / row 64, into stacked ctxTn.
                    # Copy PSUM->SBUF first so the accumulator bank frees
                    # immediately and the recip/bcast/mul chain runs off the
                    # PE critical path.
                    bcs = []
                    ctxss = []
                    for h in range(HPC):
                        dn = nrm.tile([1, 512], F32, tag=f"dn{h}",
                                      name=f"dn{h}")
                        nc.vector.tensor_copy(dn, ctx[h][D:D + 1, :])
                        ctxs = csb.tile([D, 512], F32, tag=f"ctxs{h}",
                                        name=f"ctxs{h}")
                        nc.vector.tensor_copy(ctxs, ctx[h][0:D, :])
                        rc = nrm.tile([1, 512], F32, tag=f"rc{h}",
                                      name=f"rc{h}")
                        nc.vector.reciprocal_approx_fast(rc, dn)
                        bc = nrm.tile([D, 512], F32, tag=f"bc{h}",
                                      name=f"bc{h}")
                        nc.gpsimd.partition_broadcast(bc, rc)
                        bcs.append(bc)
                        ctxss.append(ctxs)

                    def emit_mul(h, lo, hi):
                        # h0's multiply runs on GpSimd (idle engine, no
                        # partition shift); h1's needs the base-64 write,
                        # keep it on DVE where that is proven.
                        eng = nc.gpsimd if h == 0 else nc.vector
                        eng.tensor_mul(
                            out=ctxTn[h * D:(h + 1) * D, q0 + lo:q0 + hi],
                            in0=ctxss[h][:, lo:hi], in1=bcs[h][:, lo:hi])

                    def emit_op(t4, nch2, po, ot_slice):
                        tok = q0 + t4 * 128
                        nc.tensor.matmul(
                            out=po,
                            lhsT=ctxTn[:, tok:tok + 128],
                            rhs=wo_sb[:, nch2 * 512:(nch2 + 1) * 512],
                            start=True, stop=True)
                        nc.vector.tensor_copy(ot_slice, po)

                    if not last:
                        for h in range(HPC):
                            emit_mul(h, 0, 512)
                        for nch2 in range(2):
                            ot = osb.tile([128, 4, 512], F32, tag="ot",
                                          name="ot")
                            for t4 in range(4):
                                po = psO.tile([128, 512], F32, tag="po",
                                              name="po")
                                emit_op(t4, nch2, po, ot[:, t4, :])
                            nc.sync.dma_start(
                                out=out[q0:q0 + 512,
                                        nch2 * 512:(nch2 + 1) * 512].rearrange(
                                            "(t p) c -> p t c", p=128),
                                in_=ot)
                    else:
                        # final chunk: per-qtile pipeline to shorten the tail
                        for t4 in range(4):
                            for h in range(HPC):
                                emit_mul(h, t4 * 128, (t4 + 1) * 128)
                            for nch2 in range(2):
                                po = psS.tile([128, 512], F32, tag="s",
                                              name="po")
                                ot = osb.tile([128, 512], F32, tag="otl",
                                              name="otl", bufs=4)
                                emit_op(t4, nch2, po, ot)
                                tok = q0 + t4 * 128
                                nc.sync.dma_start(
                                    out=out[tok:tok + 128,
                                            nch2 * 512:(nch2 + 1) * 512],
                                    in_=ot)

                for nch in range(4):
                    emit_proj("wv", VT, nch, act_bias=True)
                for nch in range(4):
                    emit_proj("wk", KTt, nch, act_bias=True)
                emit_proj("wq", QT, 0, act_bias=True)
                # all of b1's K/V must be emitted before b1's first chunk
                # (Tile deps come from program order); spread them as filler
                # over b0's four chunks.
                fill = [[("wv", VT, 4), ("wk", KTt, 4)],
                        [("wv", VT, 5), ("wk", KTt, 5)],
                        [("wv", VT, 6), ("wk", KTt, 6)],
                        [("wv", VT, 7), ("wk", KTt, 7)],
                        [], [], [], []]
                for i in range(8):
                    if i + 1 < 8:
                        emit_proj("wq", QT, i + 1)
                    emit_attention(i // 4, i % 4, last=(i == 7))
                    for f in fill[i]:
                        emit_proj(*f)
    return nc


_NC_CACHE = None


def _get_nc():
    global _NC_CACHE
    if _NC_CACHE is None:
        nc = bacc.Bacc("TRN2", target_bir_lowering=False)
        build_core_program(nc)
        nc.finalize()
        _NC_CACHE = nc
    return _NC_CACHE


def make_in_maps(x, Wq, bq, Wk, bk, Wv, bv, Wo):
    bf = ml_dtypes.bfloat16
    x = np.asarray(x, np.float32).reshape(T, C)
    xT_bf = np.ascontiguousarray(x.T).astype(bf)
    iden = np.eye(128, dtype=bf)
    Wq = np.asarray(Wq, np.float32)
    Wk = np.asarray(Wk, np.float32)
    Wv = np.asarray(Wv, np.float32)
    Wo = np.asarray(Wo, np.float32)
    bq = np.asarray(bq, np.float32)
    bk = np.asarray(bk, np.float32)
    bv = np.asarray(bv, np.float32)
    in_maps = []
    for cidx in range(8):
        hs = slice(cidx * DPC, (cidx + 1) * DPC)
        in_maps.append(dict(
            xT=xT_bf,
            wq=np.ascontiguousarray(Wq[:, hs] * 0.125).astype(bf),
            wk=np.ascontiguousarray(Wk[:, hs]).astype(bf),
            wv=np.ascontiguousarray(Wv[:, hs]).astype(bf),
            wo=np.ascontiguousarray(Wo[hs, :]).astype(bf),
            bqkv=np.stack([bq[hs] * 0.125, bk[hs], bv[hs]],
                          axis=1).astype(np.float32),
            iden=iden,
        ))
    return in_maps


def kernel(x, Wq, bq, Wk, bk, Wv, bv, Wo, bo, _trace=False, _trace_kwargs=None):
    in_maps = make_in_maps(x, Wq, bq, Wk, bk, Wv, bv, Wo)
    nc = _get_nc()
    res = run_bass_kernel_spmd(
        nc, in_maps, core_ids=list(range(8)),
        trace=_trace, **(_trace_kwargs or {}))
    acc = res.results[0]["out"].copy()
    for cidx in range(1, 8):
        acc += res.results[cidx]["out"]
    acc += np.asarray(bo, np.float32)[None, :]
    out = acc.reshape(B, N, C)
    kernel.last_results = res
    return out


# revision 24
# speedup vs baseline: 1.0839x; 1.0839x over previous
"""Multi-head attention (B=2, N=2048, C=1024, H=16, D=64) on 8 TRN2 cores.

Sharding: tensor-parallel over heads — 2 heads per core. Each core computes
Q/K/V projections for its 2 heads, attention, and a partial output
projection (its heads' slice of Wo). Host sums the 8 partial outputs + bo.

Per-core dataflow (all matmul inputs bf16, PSUM accumulation fp32):
  xT [1024, 4096] (x transposed on host, replicated to all cores)
  QT/KT = W.T @ x.T   -> [128 (2 heads x 64), 4096]  (lhsT=W chunk, rhs=xT)
  VT likewise, then PE-transposed into v_aug [keys, 65] per head
  (65th column = ones -> softmax denominator comes out of the ctx matmul)
  S^T = K @ Q.T  -> [keys, q] in PSUM; exp on ScalarE -> bf16 SBUF
  ctx^T_aug [65, q] = v_aug.T @ expS^T  (row 64 = denominator)
  normalize: recip(row 64), gpsimd partition_broadcast, DVE multiply
  out_partial [4096, 1024] = ctx^T.T @ Wo_slice  (fp32 out, summed on host)

The 1/sqrt(D) scale is folded into Wq/bq on the host (exact: 0.125).
"""

import numpy as np
import ml_dtypes

import concourse.bass as bass
from concourse import bacc
import concourse.tile as tile
from concourse import mybir, library_config
from concourse.bass_utils import run_bass_kernel_spmd

BF16 = mybir.dt.bfloat16
F32 = mybir.dt.float32

B, N, C = 2, 2048, 1024
H, D = 16, 64
T = B * N              # 4096 tokens
HPC = H // 8           # heads per core = 2
DPC = HPC * D          # head dims per core = 128


def build_core_program(nc):
    """Emit the per-core SPMD program. Same program on all 8 cores;
    per-core data differences come from the input maps."""
    xT = nc.dram_tensor("xT", [C, T], BF16, kind="ExternalInput").ap()
    wq = nc.dram_tensor("wq", [C, DPC], BF16, kind="ExternalInput").ap()
    wk = nc.dram_tensor("wk", [C, DPC], BF16, kind="ExternalInput").ap()
    wv = nc.dram_tensor("wv", [C, DPC], BF16, kind="ExternalInput").ap()
    wo = nc.dram_tensor("wo", [DPC, C], BF16, kind="ExternalInput").ap()
    bqkv = nc.dram_tensor("bqkv", [DPC, 3], F32, kind="ExternalInput").ap()
    iden = nc.dram_tensor("iden", [128, 128], BF16, kind="ExternalInput").ap()
    out = nc.dram_tensor("out", [T, C], F32, kind="ExternalOutput").ap()

    KCH = C // 128     # 8 contraction chunks for projections
    NCH = T // 512     # 8 token chunks of 512
    KT16 = N // 128    # 16 key tiles per batch

    with tile.TileContext(nc) as tc:
        with tc.tile_pool(name="singles", bufs=1) as singles:
            nc.gpsimd.load_library(library_config.proxy)

            id_sb = singles.tile([128, 128], BF16, tag="iden")
            nc.sync.dma_start(out=id_sb, in_=iden)

            w_sb = {}
            for nm, w in (("wq", wq), ("wk", wk), ("wv", wv)):
                t = singles.tile([128, KCH, DPC], BF16, tag=f"w{nm}",
                                 name=f"w{nm}")
                nc.sync.dma_start(
                    out=t, in_=w.rearrange("(k p) j -> p k j", p=128))
                w_sb[nm] = [t[:, k, :] for k in range(KCH)]
            wo_sb = singles.tile([DPC, C], BF16, tag="wo")
            nc.sync.dma_start(out=wo_sb, in_=wo)

            bqkv_sb = singles.tile([DPC, 3], F32, tag="bqkv")
            nc.sync.dma_start(out=bqkv_sb, in_=bqkv)
            b_sb = {"bq": bqkv_sb[:, 0:1], "bk": bqkv_sb[:, 1:2],
                    "bv": bqkv_sb[:, 2:3]}

            # xT resident in SBUF as 8x2 tiles [128, 2048]; chunk 0 for all
            # feature rows first so the first projection starts after ~4MB.
            xt = [[singles.tile([128, 2048], BF16, tag=f"xt{k}_{c}", name=f"xt{k}_{c}")
                   for c in range(2)] for k in range(KCH)]
            for c in range(2):
                for k in range(KCH):
                    nc.sync.dma_start(
                        out=xt[k][c],
                        in_=xT[k * 128:(k + 1) * 128, c * 2048:(c + 1) * 2048])

            QT = singles.tile([128, T], BF16, tag="QT")
            KTt = singles.tile([128, T], BF16, tag="KT")
            VT = singles.tile([128, T], BF16, tag="VT")
            ctxTn = singles.tile([128, T], BF16, tag="ctxTn")
            vaug = [[singles.tile([128, KT16, D + 1], BF16, tag=f"vaug{b}{h}", name=f"vaug{b}{h}")
                     for h in range(HPC)] for b in range(B)]
            for b in range(B):
                for h in range(HPC):
                    nc.vector.memset(vaug[b][h], 1.0)

            # One unified PSUM layout for the whole kernel so projections and
            # attention can overlap freely (8 banks: pj 2 + s 4 + ctx 2).
            # Emission order interleaves per-batch: V+transposes and K for a
            # batch, then per q-chunk the matching Q projection followed by
            # that chunk's attention — later projections fill PE bubbles
            # while ACT works through the exps.
            with tc.tile_pool(name="psP", bufs=1, space="PSUM") as psP, \
                    tc.tile_pool(name="psO", bufs=1, space="PSUM") as psO, \
                    tc.tile_pool(name="psS", bufs=2, space="PSUM") as psS, \
                    tc.tile_pool(name="psC", bufs=1, space="PSUM") as psC, \
                    tc.tile_pool(name="esb", bufs=6) as esb, \
                    tc.tile_pool(name="nrm", bufs=2) as nrm, \
                    tc.tile_pool(name="csb", bufs=2) as csb, \
                    tc.tile_pool(name="osb", bufs=3) as osb:

                # keep PE busy (HAM warm) while xT streams in
                for wu in range(16):
                    ptw = psS.tile([128, 128], BF16, tag="s", name="ptw")
                    nc.tensor.transpose(ptw, id_sb, id_sb)
                proj_ctr = [0]

                def emit_proj(nm, dstT, nch, act_bias=False):
                    # alternate between the two single-bank tags so one
                    # chain's bias drain overlaps the next chain's matmuls
                    pool, tg = ((psP, "pj") if proj_ctr[0] % 2 == 0
                                else (psO, "po"))
                    proj_ctr[0] += 1
                    ps = pool.tile([128, 512], F32, tag=tg, name="pj")
                    c, off = divmod(nch * 512, 2048)
                    for k in range(KCH):
                        nc.tensor.matmul(
                            out=ps, lhsT=w_sb[nm][k],
                            rhs=xt[k][c][:, off:off + 512],
                            start=(k == 0), stop=(k == KCH - 1))
                    dst = dstT[:, nch * 512:(nch + 1) * 512]
                    if act_bias:
                        # ScalarE is idle before attention starts; using it
                        # for the bias keeps the single pj bank turning over
                        # without waiting on the DVE queue.
                        nc.scalar.activation(
                            out=dst, in_=ps,
                            func=mybir.ActivationFunctionType.Identity,
                            bias=b_sb["b" + nm[1]], scale=1.0)
                    else:
                        nc.vector.tensor_scalar_add(
                            out=dst, in0=ps, scalar1=b_sb["b" + nm[1]])
                    if nm == "wv":
                        # transpose the 4 just-projected 128-token tiles of V
                        # into v_aug [keys, 65] per head
                        for t16 in range(nch * 4, nch * 4 + 4):
                            b, bt = divmod(t16, KT16)
                            pt = psO.tile([128, 128], BF16, tag="po",
                                          name="pt")
                            base = t16 * 128
                            nc.tensor.transpose(
                                pt, VT[:, base:base + 128], id_sb)
                            nc.vector.tensor_copy(
                                out=vaug[b][0][:, bt, 0:D], in_=pt[:, 0:D])
                            nc.vector.tensor_copy(
                                out=vaug[b][1][:, bt, 0:D], in_=pt[:, D:2 * D])

                def emit_attention(b, qch, last=False):
                    q0 = b * N + qch * 512
                    ctx = [psC.tile([D + 1, 512], F32, tag=f"ctx{h}",
                                    name=f"ctx{h}") for h in range(HPC)]
                    for kc in range(KT16):
                        k0 = b * N + kc * 128
                        pS = psS.tile([128, 1024], F32, tag="s", name="s")
                        for h in range(HPC):
                            nc.tensor.matmul(
                                out=pS[:, h * 512:(h + 1) * 512],
                                lhsT=KTt[h * D:(h + 1) * D, k0:k0 + 128],
                                rhs=QT[h * D:(h + 1) * D, q0:q0 + 512],
                                start=True, stop=True)
                        eS = esb.tile([128, 1024], BF16, tag="e", name="e")
                        nc.scalar.activation(
                            eS, pS, mybir.ActivationFunctionType.Exp)
                        for h in range(HPC):
                            nc.tensor.matmul(
                                out=ctx[h],
                                lhsT=vaug[b][h][:, kc, :],
                                rhs=eS[:, h * 512:(h + 1) * 512],
                                start=(kc == 0), stop=(kc == KT16 - 1))
                    # normalize: rows 0..63 / row 64, into stacked ctxTn.
                    # Copy PSUM->SBUF first so the accumulator bank frees
                    # immediately and the recip/bcast/mul chain runs off the
                    # PE critical path.
                    bcs = []
                    ctxss = []
                    for h in range(HPC):
                        dn = nrm.tile([1, 512], F32, tag=f"dn{h}",
                                      name=f"dn{h}")
                        nc.vector.tensor_copy(dn, ctx[h][D:D + 1, :])
                        ctxs = csb.tile([D, 512], F32, tag=f"ctxs{h}",
                                        name=f"ctxs{h}")
                        nc.vector.tensor_copy(ctxs, ctx[h][0:D, :])
                        rc = nrm.tile([1, 512], F32, tag=f"rc{h}",
                                      name=f"rc{h}")
                        nc.vector.reciprocal_approx_fast(rc, dn)
                        bc = nrm.tile([D, 512], F32, tag=f"bc{h}",
                                      name=f"bc{h}")
                        nc.gpsimd.partition_broadcast(bc, rc)
                        bcs.append(bc)
                        ctxss.append(ctxs)

                    def emit_mul(h, lo, hi):
                        # h0's multiply runs on GpSimd (idle engine, no
                        # partition shift); h1's needs the base-64 write,
                        # keep it on DVE where that is proven.
                        eng = nc.gpsimd if h == 0 else nc.vector
                        eng.tensor_mul(
                            out=ctxTn[h * D:(h + 1) * D, q0 + lo:q0 + hi],
                            in0=ctxss[h][:, lo:hi], in1=bcs[h][:, lo:hi])

                    def emit_op(t4, nch2, po, ot_slice):
                        tok = q0 + t4 * 128
                        nc.tensor.matmul(
                            out=po,
                            lhsT=ctxTn[:, tok:tok + 128],
                            rhs=wo_sb[:, nch2 * 512:(nch2 + 1) * 512],
                            start=True, stop=True)
                        nc.vector.tensor_copy(ot_slice, po)

                    if not last:
                        for h in range(HPC):
                            emit_mul(h, 0, 512)
                        for nch2 in range(2):
                            ot = osb.tile([128, 4, 512], F32, tag="ot",
                                          name="ot")
                            for t4 in range(4):
                                po = psO.tile([128, 512], F32, tag="po",
                                              name="po")
                                emit_op(t4, nch2, po, ot[:, t4, :])
                            nc.sync.dma_start(
                                out=out[q0:q0 + 512,
                                        nch2 * 512:(nch2 + 1) * 512].rearrange(
                                            "(t p) c -> p t c", p=128),
                                in_=ot)
                    else:
                        # final chunk: per-qtile pipeline to shorten the tail
                        for t4 in range(4):
                            for h in range(HPC):
                                emit_mul(h, t4 * 128, (t4 + 1) * 128)
                            for nch2 in range(2):
                                po = psS.tile([128, 512], F32, tag="s",
                                              name="po")
                                ot = osb.tile([128, 512], F32, tag="otl",
                                              name="otl", bufs=4)
                                emit_op(t4, nch2, po, ot)
                                tok = q0 + t4 * 128
                                nc.sync.dma_start(
                                    out=out[tok:tok + 128,
                                            nch2 * 512:(nch2 + 1) * 512],
                                    in_=ot)

                for nm, dstT, nch in (("wv", VT, 0), ("wk", KTt, 0),
                                      ("wq", QT, 0), ("wk", KTt, 1),
                                      ("wk", KTt, 2), ("wk", KTt, 3),
                                      ("wv", VT, 1), ("wv", VT, 2),
                                      ("wv", VT, 3)):
                    emit_proj(nm, dstT, nch, act_bias=True)
                # all of b1's K/V must be emitted before b1's first chunk
                # (Tile deps come from program order); spread them as filler
                # over b0's four chunks.
                fill = [[("wv", VT, 4), ("wk", KTt, 4)],
                        [("wv", VT, 5), ("wk", KTt, 5)],
                        [("wv", VT, 6), ("wk", KTt, 6)],
                        [("wv", VT, 7), ("wk", KTt, 7)],
                        [], [], [], []]
                for i in range(8):
                    if i + 1 < 8:
                        emit_proj("wq", QT, i + 1)
                    emit_attention(i // 4, i % 4, last=(i == 7))
                    for f in fill[i]:
                        emit_proj(*f)
    return nc


_NC_CACHE = None


def _get_nc():
    global _NC_CACHE
    if _NC_CACHE is None:
        nc = bacc.Bacc("TRN2", target_bir_lowering=False)
        build_core_program(nc)
        nc.finalize()
        _NC_CACHE = nc
    return _NC_CACHE


def make_in_maps(x, Wq, bq, Wk, bk, Wv, bv, Wo):
    bf = ml_dtypes.bfloat16
    x = np.asarray(x, np.float32).reshape(T, C)
    xT_bf = np.ascontiguousarray(x.T).astype(bf)
    iden = np.eye(128, dtype=bf)
    Wq = np.asarray(Wq, np.float32)
    Wk = np.asarray(Wk, np.float32)
    Wv = np.asarray(Wv, np.float32)
    Wo = np.asarray(Wo, np.float32)
    bq = np.asarray(bq, np.float32)
    bk = np.asarray(bk, np.float32)
    bv = np.asarray(bv, np.float32)
    in_maps = []
    for cidx in range(8):
        hs = slice(cidx * DPC, (cidx + 1) * DPC)
        in_maps.append(dict(
            xT=xT_bf,
            wq=np.ascontiguousarray(Wq[:, hs] * 0.125).astype(bf),
            wk=np.ascontiguousarray(Wk[:, hs]).astype(bf),
            wv=np.ascontiguousarray(Wv[:, hs]).astype(bf),
            wo=np.ascontiguousarray(Wo[hs, :]).astype(bf),
            bqkv=np.stack([bq[hs] * 0.125, bk[hs], bv[hs]],
                          axis=1).astype(np.float32),
            iden=iden,
        ))
    return in_maps


def kernel(x, Wq, bq, Wk, bk, Wv, bv, Wo, bo, _trace=False, _trace_kwargs=None):
    in_maps = make_in_maps(x, Wq, bq, Wk, bk, Wv, bv, Wo)
    nc = _get_nc()
    res = run_bass_kernel_spmd(
        nc, in_maps, core_ids=list(range(8)),
        trace=_trace, **(_trace_kwargs or {}))
    acc = res.results[0]["out"].copy()
    for cidx in range(1, 8):
        acc += res.results[cidx]["out"]
    acc += np.asarray(bo, np.float32)[None, :]
    out = acc.reshape(B, N, C)
    kernel.last_results = res
    return out


# revision 26
# speedup vs baseline: 1.1000x; 1.0149x over previous
"""Multi-head attention (B=2, N=2048, C=1024, H=16, D=64) on 8 TRN2 cores.

Sharding: tensor-parallel over heads — 2 heads per core. Each core computes
Q/K/V projections for its 2 heads, attention, and a partial output
projection (its heads' slice of Wo). Host sums the 8 partial outputs + bo.

Per-core dataflow (all matmul inputs bf16, PSUM accumulation fp32):
  xT [1024, 4096] (x transposed on host, replicated to all cores)
  QT/KT = W.T @ x.T   -> [128 (2 heads x 64), 4096]  (lhsT=W chunk, rhs=xT)
  VT likewise, then PE-transposed into v_aug [keys, 65] per head
  (65th column = ones -> softmax denominator comes out of the ctx matmul)
  S^T = K @ Q.T  -> [keys, q] in PSUM; exp on ScalarE -> bf16 SBUF
  ctx^T_aug [65, q] = v_aug.T @ expS^T  (row 64 = denominator)
  normalize: recip(row 64), gpsimd partition_broadcast, DVE multiply
  out_partial [4096, 1024] = ctx^T.T @ Wo_slice  (fp32 out, summed on host)

The 1/sqrt(D) scale is folded into Wq/bq on the host (exact: 0.125).
"""

import numpy as np
import ml_dtypes

import concourse.bass as bass
from concourse import bacc
import concourse.tile as tile
from concourse import mybir, library_config
from concourse.bass_utils import run_bass_kernel_spmd

BF16 = mybir.dt.bfloat16
F32 = mybir.dt.float32

B, N, C = 2, 2048, 1024
H, D = 16, 64
T = B * N              # 4096 tokens
HPC = H // 8           # heads per core = 2
DPC = HPC * D          # head dims per core = 128


def build_core_program(nc):
    """Emit the per-core SPMD program. Same program on all 8 cores;
    per-core data differences come from the input maps."""
    xT = nc.dram_tensor("xT", [C, T], BF16, kind="ExternalInput").ap()
    wq = nc.dram_tensor("wq", [C, DPC], BF16, kind="ExternalInput").ap()
    wk = nc.dram_tensor("wk", [C, DPC], BF16, kind="ExternalInput").ap()
    wv = nc.dram_tensor("wv", [C, DPC], BF16, kind="ExternalInput").ap()
    wo = nc.dram_tensor("wo", [DPC, C], BF16, kind="ExternalInput").ap()
    bqkv = nc.dram_tensor("bqkv", [DPC, 3], F32, kind="ExternalInput").ap()
    iden = nc.dram_tensor("iden", [128, 128], BF16, kind="ExternalInput").ap()
    out = nc.dram_tensor("out", [T, C], F32, kind="ExternalOutput").ap()

    KCH = C // 128     # 8 contraction chunks for projections
    NCH = T // 512     # 8 token chunks of 512
    KT16 = N // 128    # 16 key tiles per batch

    with tile.TileContext(nc) as tc:
        with tc.tile_pool(name="singles", bufs=1) as singles:
            nc.gpsimd.load_library(library_config.proxy)

            id_sb = singles.tile([128, 128], BF16, tag="iden")
            nc.sync.dma_start(out=id_sb, in_=iden)

            w_sb = {}
            for nm, w in (("wq", wq), ("wk", wk), ("wv", wv)):
                t = singles.tile([128, KCH, DPC], BF16, tag=f"w{nm}",
                                 name=f"w{nm}")
                nc.sync.dma_start(
                    out=t, in_=w.rearrange("(k p) j -> p k j", p=128))
                w_sb[nm] = [t[:, k, :] for k in range(KCH)]
            wo_sb = singles.tile([DPC, C], BF16, tag="wo")
            nc.sync.dma_start(out=wo_sb, in_=wo)

            bqkv_sb = singles.tile([DPC, 3], F32, tag="bqkv")
            nc.sync.dma_start(out=bqkv_sb, in_=bqkv)
            b_sb = {"bq": bqkv_sb[:, 0:1], "bk": bqkv_sb[:, 1:2],
                    "bv": bqkv_sb[:, 2:3]}

            # xT resident in SBUF as 8x2 tiles [128, 2048]; chunk 0 for all
            # feature rows first so the first projection starts after ~4MB.
            xt = [[singles.tile([128, 2048], BF16, tag=f"xt{k}_{c}", name=f"xt{k}_{c}")
                   for c in range(2)] for k in range(KCH)]
            for c in range(2):
                for k in range(KCH):
                    nc.sync.dma_start(
                        out=xt[k][c],
                        in_=xT[k * 128:(k + 1) * 128, c * 2048:(c + 1) * 2048])

            QT = singles.tile([128, T], BF16, tag="QT")
            KTt = singles.tile([128, T], BF16, tag="KT")
            VT = singles.tile([128, T], BF16, tag="VT")
            ctxTn = singles.tile([128, T], BF16, tag="ctxTn")
            vaug = [[singles.tile([128, KT16, D + 1], BF16, tag=f"vaug{b}{h}", name=f"vaug{b}{h}")
                     for h in range(HPC)] for b in range(B)]
            for b in range(B):
                for h in range(HPC):
                    nc.vector.memset(vaug[b][h], 1.0)

            # One unified PSUM layout for the whole kernel so projections and
            # attention can overlap freely (8 banks: pj 2 + s 4 + ctx 2).
            # Emission order interleaves per-batch: V+transposes and K for a
            # batch, then per q-chunk the matching Q projection followed by
            # that chunk's attention — later projections fill PE bubbles
            # while ACT works through the exps.
            with tc.tile_pool(name="psP", bufs=1, space="PSUM") as psP, \
                    tc.tile_pool(name="psO", bufs=1, space="PSUM") as psO, \
                    tc.tile_pool(name="psS", bufs=2, space="PSUM") as psS, \
                    tc.tile_pool(name="psC", bufs=1, space="PSUM") as psC, \
                    tc.tile_pool(name="esb", bufs=6) as esb, \
                    tc.tile_pool(name="nrm", bufs=3) as nrm, \
                    tc.tile_pool(name="csb", bufs=3) as csb, \
                    tc.tile_pool(name="osb", bufs=3) as osb:

                # keep PE busy (HAM warm) while xT streams in
                for wu in range(16):
                    ptw = psS.tile([128, 128], BF16, tag="s", name="ptw")
                    nc.tensor.transpose(ptw, id_sb, id_sb)
                proj_ctr = [0]

                def emit_proj(nm, dstT, nch, act_bias=False):
                    # alternate between the two single-bank tags so one
                    # chain's bias drain overlaps the next chain's matmuls
                    pool, tg = ((psP, "pj") if proj_ctr[0] % 2 == 0
                                else (psO, "po"))
                    proj_ctr[0] += 1
                    ps = pool.tile([128, 512], F32, tag=tg, name="pj")
                    c, off = divmod(nch * 512, 2048)
                    for k in range(KCH):
                        nc.tensor.matmul(
                            out=ps, lhsT=w_sb[nm][k],
                            rhs=xt[k][c][:, off:off + 512],
                            start=(k == 0), stop=(k == KCH - 1))
                    dst = dstT[:, nch * 512:(nch + 1) * 512]
                    if act_bias:
                        # ScalarE is idle before attention starts; using it
                        # for the bias keeps the single pj bank turning over
                        # without waiting on the DVE queue.
                        nc.scalar.activation(
                            out=dst, in_=ps,
                            func=mybir.ActivationFunctionType.Identity,
                            bias=b_sb["b" + nm[1]], scale=1.0)
                    else:
                        nc.vector.tensor_scalar_add(
                            out=dst, in0=ps, scalar1=b_sb["b" + nm[1]])
                    if nm == "wv":
                        # transpose the 4 just-projected 128-token tiles of V
                        # into v_aug [keys, 65] per head
                        for t16 in range(nch * 4, nch * 4 + 4):
                            b, bt = divmod(t16, KT16)
                            pt = psO.tile([128, 128], BF16, tag="po",
                                          name="pt")
                            base = t16 * 128
                            nc.tensor.transpose(
                                pt, VT[:, base:base + 128], id_sb)
                            nc.vector.tensor_copy(
                                out=vaug[b][0][:, bt, 0:D], in_=pt[:, 0:D])
                            nc.vector.tensor_copy(
                                out=vaug[b][1][:, bt, 0:D], in_=pt[:, D:2 * D])

                def emit_attention(b, qch, last=False):
                    q0 = b * N + qch * 512
                    ctx = [psC.tile([D + 1, 512], F32, tag=f"ctx{h}",
                                    name=f"ctx{h}") for h in range(HPC)]
                    for kc in range(KT16):
                        k0 = b * N + kc * 128
                        pS = psS.tile([128, 1024], F32, tag="s", name="s")
                        for h in range(HPC):
                            nc.tensor.matmul(
                                out=pS[:, h * 512:(h + 1) * 512],
                                lhsT=KTt[h * D:(h + 1) * D, k0:k0 + 128],
                                rhs=QT[h * D:(h + 1) * D, q0:q0 + 512],
                                start=True, stop=True)
                        eS = esb.tile([128, 1024], BF16, tag="e", name="e")
                        nc.scalar.activation(
                            eS, pS, mybir.ActivationFunctionType.Exp)
                        for h in range(HPC):
                            nc.tensor.matmul(
                                out=ctx[h],
                                lhsT=vaug[b][h][:, kc, :],
                                rhs=eS[:, h * 512:(h + 1) * 512],
                                start=(kc == 0), stop=(kc == KT16 - 1))
                    # normalize: rows 0..63 / row 64, into stacked ctxTn.
                    # Copy PSUM->SBUF first so the accumulator bank frees
                    # immediately and the recip/bcast/mul chain runs off the
                    # PE critical path.
                    bcs = []
                    ctxss = []
                    for h in range(HPC):
                        dn = nrm.tile([1, 512], F32, tag=f"dn{h}",
                                      name=f"dn{h}")
                        nc.vector.tensor_copy(dn, ctx[h][D:D + 1, :])
                        ctxs = csb.tile([D, 512], F32, tag=f"ctxs{h}",
                                        name=f"ctxs{h}")
                        nc.vector.tensor_copy(ctxs, ctx[h][0:D, :])
                        rc = nrm.tile([1, 512], F32, tag=f"rc{h}",
                                      name=f"rc{h}")
                        nc.vector.reciprocal_approx_fast(rc, dn)
                        bc = nrm.tile([D, 512], F32, tag=f"bc{h}",
                                      name=f"bc{h}")
                        nc.gpsimd.partition_broadcast(bc, rc)
                        bcs.append(bc)
                        ctxss.append(ctxs)

                    def emit_mul(h, lo, hi):
                        # h0's multiply runs on GpSimd (idle engine, no
                        # partition shift); h1's needs the base-64 write,
                        # keep it on DVE where that is proven.
                        eng = nc.gpsimd if h == 0 else nc.vector
                        eng.tensor_mul(
                            out=ctxTn[h * D:(h + 1) * D, q0 + lo:q0 + hi],
                            in0=ctxss[h][:, lo:hi], in1=bcs[h][:, lo:hi])

                    def emit_op(t4, nch2, po, ot_slice):
                        tok = q0 + t4 * 128
                        nc.tensor.matmul(
                            out=po,
                            lhsT=ctxTn[:, tok:tok + 128],
                            rhs=wo_sb[:, nch2 * 512:(nch2 + 1) * 512],
                            start=True, stop=True)
                        nc.vector.tensor_copy(ot_slice, po)

                    if not last:
                        for h in range(HPC):
                            emit_mul(h, 0, 512)

                    def emit_outproj():
                        for nch2 in range(2):
                            ot = osb.tile([128, 4, 512], F32, tag="ot",
                                          name="ot")
                            for t4 in range(4):
                                po = psO.tile([128, 512], F32, tag="po",
                                              name="po")
                                emit_op(t4, nch2, po, ot[:, t4, :])
                            nc.sync.dma_start(
                                out=out[q0:q0 + 512,
                                        nch2 * 512:(nch2 + 1) * 512].rearrange(
                                            "(t p) c -> p t c", p=128),
                                in_=ot)

                    if not last:
                        return emit_outproj
                    else:
                        # final chunk: per-qtile pipeline to shorten the tail
                        for t4 in range(4):
                            for h in range(HPC):
                                emit_mul(h, t4 * 128, (t4 + 1) * 128)
                            for nch2 in range(2):
                                po = psS.tile([128, 512], F32, tag="s",
                                              name="po")
                                ot = osb.tile([128, 512], F32, tag="otl",
                                              name="otl", bufs=4)
                                emit_op(t4, nch2, po, ot)
                                tok = q0 + t4 * 128
                                nc.sync.dma_start(
                                    out=out[tok:tok + 128,
                                            nch2 * 512:(nch2 + 1) * 512],
                                    in_=ot)
                    return None

                for nm, dstT, nch in (("wv", VT, 0), ("wk", KTt, 0),
                                      ("wq", QT, 0), ("wk", KTt, 1),
                                      ("wk", KTt, 2), ("wk", KTt, 3),
                                      ("wv", VT, 1), ("wv", VT, 2),
                                      ("wv", VT, 3)):
                    emit_proj(nm, dstT, nch, act_bias=True)
                # all of b1's K/V must be emitted before b1's first chunk
                # (Tile deps come from program order); spread them as filler
                # over b0's four chunks.
                fill = [[("wv", VT, 4), ("wk", KTt, 4)],
                        [("wv", VT, 5), ("wk", KTt, 5)],
                        [("wv", VT, 6), ("wk", KTt, 6)],
                        [("wv", VT, 7), ("wk", KTt, 7)],
                        [], [], [], []]
                pending_op = None
                for i in range(8):
                    if i + 1 < 8:
                        emit_proj("wq", QT, i + 1)
                    op_i = emit_attention(i // 4, i % 4, last=(i == 7))
                    if pending_op is not None:
                        pending_op()
                    if i == 6 and op_i is not None:
                        op_i()
                        op_i = None
                    pending_op = op_i
                    for f in fill[i]:
                        emit_proj(*f)
    return nc


_NC_CACHE = None


def _get_nc():
    global _NC_CACHE
    if _NC_CACHE is None:
        nc = bacc.Bacc("TRN2", target_bir_lowering=False)
        build_core_program(nc)
        nc.finalize()
        _NC_CACHE = nc
    return _NC_CACHE


def make_in_maps(x, Wq, bq, Wk, bk, Wv, bv, Wo):
    bf = ml_dtypes.bfloat16
    x = np.asarray(x, np.float32).reshape(T, C)
    xT_bf = np.ascontiguousarray(x.T).astype(bf)
    iden = np.eye(128, dtype=bf)
    Wq = np.asarray(Wq, np.float32)
    Wk = np.asarray(Wk, np.float32)
    Wv = np.asarray(Wv, np.float32)
    Wo = np.asarray(Wo, np.float32)
    bq = np.asarray(bq, np.float32)
    bk = np.asarray(bk, np.float32)
    bv = np.asarray(bv, np.float32)
    in_maps = []
    for cidx in range(8):
        hs = slice(cidx * DPC, (cidx + 1) * DPC)
        in_maps.append(dict(
            xT=xT_bf,
            wq=np.ascontiguousarray(Wq[:, hs] * 0.125).astype(bf),
            wk=np.ascontiguousarray(Wk[:, hs]).astype(bf),
            wv=np.ascontiguousarray(Wv[:, hs]).astype(bf),
            wo=np.ascontiguousarray(Wo[hs, :]).astype(bf),
            bqkv=np.stack([bq[hs] * 0.125, bk[hs], bv[hs]],
                          axis=1).astype(np.float32),
            iden=iden,
        ))
    return in_maps


def kernel(x, Wq, bq, Wk, bk, Wv, bv, Wo, bo, _trace=False, _trace_kwargs=None):
    in_maps = make_in_maps(x, Wq, bq, Wk, bk, Wv, bv, Wo)
    nc = _get_nc()
    res = run_bass_kernel_spmd(
        nc, in_maps, core_ids=list(range(8)),
        trace=_trace, **(_trace_kwargs or {}))
    acc = res.results[0]["out"].copy()
    for cidx in range(1, 8):
        acc += res.results[cidx]["out"]
    acc += np.asarray(bo, np.float32)[None, :]
    out = acc.reshape(B, N, C)
    kernel.last_results = res
    return out
